# revision 28
# baseline (speedup 1.0000x reference)
"""Multi-head attention (B=2, S=2048, E=1024, H=16, D=64) on 8 Trainium2 cores.

Sharding: data-parallel over batch (2 groups of 4 cores), tensor-parallel over
heads within each group (4 heads per core, Megatron-style column-split qkv and
ROW-split out_proj). Each core emits its PARTIAL out-projection
y_c = attn_out(core heads) @ w_out[rows of those heads, :]  -> [S, E]
and the host sums the 4 partials per batch during unsharding. There is no
on-device collective: the inter-core reduction is part of host-side
gather/unshard, so every engine-second on device is compute.

Per-core pipeline (all matmuls f32r = full PE rate):
  host supplies x^T (free transpose) -> q^T/k^T = (W as lhsT) @ x^T and
  v natural = (x^T as lhsT) @ Wv
  -> S^T = k q^T per head, two heads row-packed as K=64 pairs (tile_position)
  -> exp split across TWO engines:
       ScalarE chunks: activation(Exp, scale=ln2) on the log2-domain logits
         (Wq is pre-scaled by log2(e)/8 on the host, so logits arrive in
          log2 domain; no extra work anywhere)
       VectorE chunks: 2-instruction bit-trick exp2 — tensor_scalar affine
         with int32 convert-on-write (Schraudolph seed), then a custom DVE op
         (registered at import; per-NEFF uop table) that extracts the
         mantissa with AND/OR and applies a quadratic sawtooth correction.
         Max rel err ~7.5e-3 on the exp, ~1e-3 after softmax mixing.
  -> fused PV^T + denominator: lhsT = [v_h | ones] (M=128), PSUM rows 0:64
     accumulate PV^T, rows 64:128 the pre-broadcast denominator
  -> reciprocal (fast approx) + multiply -> outT
  -> row-split out_proj per query block, interleaved into the next block's
     attention so the PE never idles; partial y DMA'd out as it's produced.
"""

import numpy as np
from contextlib import ExitStack

import concourse.tile as tile
from concourse import bacc, mybir
from concourse.bass_utils import run_bass_kernel_spmd

# --------------------------------------------------------------------------
# Custom DVE op: quadratic sawtooth correction for the bit-trick exp2.
# y0 = bitcast(int32(u*2^23 + 127*2^23)) = 2^n * (1+g); this op computes
# y0 * q(r), r = 1+g in [1,2), q(r) ~= 2^(r-1)/r (rel minimax quadratic).
# --------------------------------------------------------------------------
import concourse.dve_ops as _dops
from concourse.dve_spec import (Spec as _Spec, Src0 as _Src0, C0 as _C0,
                                C1 as _C1, C2 as _C2, C3 as _C3, One as _One,
                                AluOp as _AluOp, Bin as _Bin,
                                lower as _lower, _spill_c3_to_src1,
                                _has_src1)
from concourse.dve_uop import DveOpSpec as _DveOpSpec

QC0, QC1, QC2 = 1.4517454385304476, -0.6921680514362218, 0.23303615941790978
MASK_F32 = float(np.int32(0x007FFFFF).view(np.float32))
LN2 = float(np.log(2.0))
LOG2E_8 = float(np.log2(np.e) / 8.0)   # folds the 1/sqrt(D) logit scale


def _make_exp2_op():
    name = "EXP2_SAW_FIX_ANT"
    for o in _dops.OPS:
        if o.name == name:
            return o
    a_n = _Bin(_AluOp.BITWISE_AND, _Src0, _C3)     # C3 (in1): mantissa mask
    r_n = _Bin(_AluOp.BITWISE_OR, a_n, _One)       # r = 1+g in [1,2)
    body = _spill_c3_to_src1(_Src0 * (_C0 + (_C1 + _C2 * r_n) * r_n))

    def _ref(in0, in1, s0, s1, imm2):
        y0 = np.asarray(in0, np.float32)
        aa = y0.view(np.int32) & np.int32(0x007FFFFF)
        rr = (aa | np.int32(0x3F800000)).view(np.float32)
        qq = (np.float32(s0) + (np.float32(s1) + np.float32(imm2) * rr)
              * rr).astype(np.float32)
        return (y0 * qq).astype(np.float32)

    spec = _Spec(body=body, reference=_ref)
    row = _dops._CUSTOM_DVE_ROW_BASE + len(_dops.OPS)
    shas = {}
    for ver in ("v3", "v4"):
        shas[ver] = _DveOpSpec(name=name, opcode=row,
                               uops=_lower(spec, ver=ver),
                               rd1_en=_has_src1(spec)).sha(ver)
    op = _dops.DveOp(name, spec, subdim=False, uops_sha=shas)
    _dops.OPS.append(op)
    _dops.CUSTOM_DVE_SPECS[name] = spec
    _dops._SUB_OPCODE_FOR_NAME[name] = row
    return op


EXP2_OP = _make_exp2_op()

B, S, E, H, D = 2, 2048, 1024, 16, 64
N_CORES = 8
HPC = 4            # heads per core
HD = HPC * D       # 256

F32 = mybir.dt.float32
F32R = mybir.dt.float32r
I32 = mybir.dt.int32

_cached = None


_DVE_KC = ({5, 9, 13}, {3, 7, 11})   # per-hp key chunks routed to VectorE
                                     # (kept off kc 0-2 where the DVE still
                                     #  drains the previous block's recip/mul)


def _dve_exp(hp, kc, qc):
    """Which exp units run on the Vector engine (2-inst bit trick)."""
    return kc in _DVE_KC[hp]


def build(reps=1, profile=False):
    nc = bacc.Bacc("TRN2", target_bir_lowering=False, debug=False,
                   num_devices=N_CORES)

    xT_d = nc.dram_tensor("xT", [E, S], F32R, kind="ExternalInput").ap()
    wq_d = nc.dram_tensor("wq", [E, HD], F32R, kind="ExternalInput").ap()
    wk_d = nc.dram_tensor("wk", [E, HD], F32R, kind="ExternalInput").ap()
    wv_d = nc.dram_tensor("wv", [E, HD], F32R, kind="ExternalInput").ap()
    wo_d = nc.dram_tensor("wo", [HD, E], F32R, kind="ExternalInput").ap()
    y_d = nc.dram_tensor("y", [S, E], F32, kind="ExternalOutput").ap()

    with tile.TileContext(nc) as tc, ExitStack() as ctx:
        glob = ctx.enter_context(tc.tile_pool(name="glob", bufs=1))
        qT_t = glob.tile([128, 2, S], F32R, tag="qT")    # head-pair dims x S
        kT_t = glob.tile([128, 2, S], F32R, tag="kT")
        v_t = glob.tile([128, 16, HPC, 128], F32R, tag="v")   # [v_h | ones]
        outT_t = glob.tile([128, 2, S], F32R, tag="outT")
        wo_t = glob.tile([128, 2, E], F32R, tag="wo")
        mask_t = glob.tile([128, 1], F32, tag="mask")
        ones_f = glob.tile([128, D], F32, tag="ones_f")

        nc.vector.memset(mask_t[:], MASK_F32)
        nc.gpsimd.memset(ones_f[:], 1.0)
        for sc in range(16):
            for h in range(HPC):
                nc.vector.tensor_copy(v_t[:, sc, h, 64:128], ones_f[:])

        for _rep in range(reps):
            _emit_body(nc, tc, xT_d, wq_d, wk_d, wv_d, wo_d, y_d,
                       qT_t, kT_t, v_t, outT_t, wo_t, mask_t)

    nc.compile()
    return nc


def _out_proj_chunk(nc, psum_pool, ysb_pool, outT_t, wo_t, y_d, qc, mc,
                    psum_tag="st"):
    yt = ysb_pool.tile([128, E], F32, tag="y")
    for half in range(2):
        yp = psum_pool.tile([128, 512], F32, tag=psum_tag)
        for hp in range(2):
            nc.tensor.matmul(
                yp[:],
                outT_t[:, hp, qc * 512 + mc * 128:qc * 512 + (mc + 1) * 128],
                wo_t[:, hp, half * 512:(half + 1) * 512],
                start=(hp == 0), stop=(hp == 1))
        dst = yt[:, half * 512:(half + 1) * 512]
        # engine-agnostic gap-filler: lands on whichever of ACT/DVE is idle
        # and never preempts the exp stream
        nc.any.tensor_copy(dst, yp[:])
    row = qc * 512 + mc * 128
    nc.sync.dma_start(y_d[row:row + 128, :], yt[:])


def _emit_body(nc, tc, xT_d, wq_d, wk_d, wv_d, wo_d, y_d,
               qT_t, kT_t, v_t, outT_t, wo_t, mask_t, dbg=None):
    xT_r = xT_d.rearrange("(c p) s -> p c s", p=128)

    # ---- Phase P: projections (q^T, k^T, v), streamed by 512-wide s-chunk
    with ExitStack() as pb:
        wpool = pb.enter_context(tc.tile_pool(name="wpool", bufs=1))
        xload = pb.enter_context(tc.tile_pool(name="xload", bufs=3))
        ppsum = pb.enter_context(tc.tile_pool(name="ppsum", bufs=2,
                                              space="PSUM"))
        wq_t = wpool.tile([128, 8, HD], F32R, tag="wq")
        wk_t = wpool.tile([128, 8, HD], F32R, tag="wk")
        wv_t = wpool.tile([128, 8, HD], F32R, tag="wv")
        # DMA order: get the first q-projection started as early as possible;
        # wo is not needed until the first out_proj (~80us in)
        nc.sync.dma_start(wq_t[:], wq_d.rearrange("(c p) n -> p c n", p=128))
        xcs = []
        xc = xload.tile([128, 8, 512], F32R, tag="xc")
        nc.sync.dma_start(xc[:], xT_r[:, :, 0:512])
        xcs.append(xc)
        nc.sync.dma_start(wk_t[:], wk_d.rearrange("(c p) n -> p c n", p=128))
        nc.sync.dma_start(wv_t[:], wv_d.rearrange("(c p) n -> p c n", p=128))
        for sq in range(1, 4):
            xc = xload.tile([128, 8, 512], F32R, tag="xc")
            nc.sync.dma_start(xc[:], xT_r[:, :, sq * 512:(sq + 1) * 512])
            xcs.append(xc)
        nc.sync.dma_start(wo_t[:], wo_d.rearrange("(c p) n -> p c n", p=128))

        for sq in range(4):
            xc = xcs[sq]
            for w_t, dst in ((wq_t, qT_t), (wk_t, kT_t)):
                for mc in range(2):
                    pp = ppsum.tile([128, 512], F32, tag="pp")
                    for ec in range(8):
                        nc.tensor.matmul(
                            pp[:],
                            w_t[:, ec, mc * 128:(mc + 1) * 128],
                            xc[:, ec, :],
                            start=(ec == 0), stop=(ec == 7))
                    nc.vector.tensor_copy(
                        dst[:, mc, sq * 512:(sq + 1) * 512], pp[:])
            for scl in range(4):
                sc = sq * 4 + scl
                vp = ppsum.tile([128, HD], F32, tag="vp")
                for ec in range(8):
                    nc.tensor.matmul(
                        vp[:],
                        xc[:, ec, scl * 128:(scl + 1) * 128],
                        wv_t[:, ec, :],
                        start=(ec == 0), stop=(ec == 7))
                nc.vector.tensor_copy(
                    v_t[:, sc, :, 0:64],
                    vp[:].rearrange("p (h d) -> p h d", h=HPC))

    # ---- Phase A: attention + interleaved row-split out_proj ----
    with ExitStack() as ab:
        stp = ab.enter_context(tc.tile_pool(name="stp", bufs=2, space="PSUM"))
        pvp = ab.enter_context(tc.tile_pool(name="pvp", bufs=2, space="PSUM"))
        expp = ab.enter_context(tc.tile_pool(name="expp", bufs=6))
        seedp = ab.enter_context(tc.tile_pool(name="seedp", bufs=3))
        rcp = ab.enter_context(tc.tile_pool(name="rcp", bufs=4))
        ysb = ab.enter_context(tc.tile_pool(name="ysb", bufs=3))

        def out_proj_chunk(qc, mc):
            _out_proj_chunk(nc, stp, ysb, outT_t, wo_t, y_d, qc, mc)

        for qc in range(4):
            fz0 = pvp.tile([128, 2, 512], F32, tag="pv")
            fz1 = pvp.tile([128, 2, 512], F32, tag="pv")
            fzs = [fz0, fz1]

            def emit_pv(hp, kc, ex):
                for par in range(2):
                    h = 2 * hp + par
                    nc.tensor.matmul(
                        fzs[hp][:, par, :],
                        v_t[:, kc, h, :],
                        ex[:, par * 512:(par + 1) * 512],
                        start=(kc == 0), stop=(kc == 15))

            def finish_hp(hp):
                # normalization for this head pair, as soon as its PV is done
                for par in range(2):
                    rc = rcp.tile([64, 512], F32, tag="rc")
                    nc.vector.reciprocal(rc[:], fzs[hp][64:128, par, :])
                    nc.vector.tensor_mul(
                        outT_t[par * 64:(par + 1) * 64, hp,
                               qc * 512:(qc + 1) * 512],
                        fzs[hp][0:64, par, :], rc[:])

            pend = []   # PV lagged 2 units behind S^T/exp to hide exp latency
            for kc in range(16):
                for hp in range(2):
                    st = stp.tile([128, 1024], F32, tag="st")
                    for par in range(2):     # row-packed K=64 head pair
                        lo, hi = par * 64, (par + 1) * 64
                        nc.tensor.matmul(
                            st[:, par * 512:(par + 1) * 512],
                            kT_t[lo:hi, hp, kc * 128:(kc + 1) * 128],
                            qT_t[lo:hi, hp, qc * 512:(qc + 1) * 512],
                            start=True, stop=True)
                    ex = expp.tile([128, 1024], F32R, tag="ex")
                    if _dve_exp(hp, kc, qc):
                        seed = seedp.tile([128, 1024], I32, tag="seed")
                        nc.vector.tensor_scalar(
                            seed[:], st[:],
                            8388608.0, 1065353216.0,
                            mybir.AluOpType.mult, mybir.AluOpType.add)
                        nc.vector._custom_dve(
                            EXP2_OP, out=ex[:], in0=seed[:].bitcast(F32),
                            in1=mask_t[:], s0=QC0, s1=QC1, imm2=QC2)
                    else:
                        nc.scalar.activation(
                            ex[:], st[:],
                            mybir.ActivationFunctionType.Exp, scale=LN2)
                    if dbg is not None and qc == 0 and kc == 0:
                        nc.sync.dma_start(dbg["ex"][hp], ex[:])
                    pend.append((hp, kc, ex))
                    if len(pend) > 2:
                        emit_pv(*pend.pop(0))
                if qc >= 1 and kc % 4 == 3:
                    out_proj_chunk(qc - 1, kc // 4)
            for item in pend:
                emit_pv(*item)
                if item[1] == 15:
                    finish_hp(item[0])
        for mc in range(4):
            out_proj_chunk(3, mc)
        if dbg is not None:
            nc.sync.dma_start(dbg["qT"], qT_t[:])
            nc.sync.dma_start(dbg["kT"], kT_t[:])
            nc.sync.dma_start(dbg["v"], v_t[:])
            nc.sync.dma_start(dbg["outT"], outT_t[:])


def _get_nc():
    global _cached
    if _cached is None:
        _cached = build()
    return _cached


def make_in_maps(x, w_qkv, w_out):
    x = np.asarray(x, dtype=np.float32)
    w_qkv = np.asarray(w_qkv, dtype=np.float32)
    w_out = np.asarray(w_out, dtype=np.float32)
    scale = np.float32(LOG2E_8)
    in_maps = []
    for c in range(N_CORES):
        b, r = c // 4, c % 4
        hs = r * HD
        in_maps.append({
            "xT": np.ascontiguousarray(x[b].T),
            "wq": np.ascontiguousarray(w_qkv[:, hs:hs + HD] * scale),
            "wk": np.ascontiguousarray(w_qkv[:, E + hs:E + hs + HD]),
            "wv": np.ascontiguousarray(w_qkv[:, 2 * E + hs:2 * E + hs + HD]),
            "wo": np.ascontiguousarray(w_out[hs:hs + HD, :]),
        })
    return in_maps


def assemble(results):
    y = np.zeros((B, S, E), dtype=np.float32)
    for c in range(N_CORES):
        y[c // 4] += results[c]["y"]
    return y


def kernel(x, w_qkv, w_out):
    nc = _get_nc()
    res = run_bass_kernel_spmd(nc, make_in_maps(x, w_qkv, w_out),
                               list(range(N_CORES)))
    return assemble(res.results)


# revision 33
# speedup vs baseline: 1.6061x; 1.6061x over previous
"""Multi-head attention (B=2, S=2048, E=1024, H=16, D=64) on 8 Trainium2 cores.

Sharding: data-parallel over batch (2 groups of 4 cores), tensor-parallel over
heads within each group (4 heads per core, Megatron-style column-split qkv and
ROW-split out_proj). Each core emits its PARTIAL out-projection
y_c = attn_out(core heads) @ w_out[rows of those heads, :]  -> [S, E]
and the host sums the 4 partials per batch during unsharding. There is no
on-device collective: the inter-core reduction is part of host-side
gather/unshard, so every engine-second on device is compute.

Per-core pipeline (all matmuls f32r = full PE rate):
  host supplies x^T (free transpose) -> q^T/k^T = (W as lhsT) @ x^T and
  v natural = (x^T as lhsT) @ Wv
  -> S^T = k q^T per head, two heads row-packed as K=64 pairs (tile_position)
  -> exp split across TWO engines:
       ScalarE chunks: activation(Exp, scale=ln2) on the log2-domain logits
         (Wq is pre-scaled by log2(e)/8 on the host, so logits arrive in
          log2 domain; no extra work anywhere)
       VectorE chunks: 2-instruction bit-trick exp2 — tensor_scalar affine
         with int32 convert-on-write (Schraudolph seed), then a custom DVE op
         (registered at import; per-NEFF uop table) that extracts the
         mantissa with AND/OR and applies a quadratic sawtooth correction.
         Max rel err ~7.5e-3 on the exp, ~1e-3 after softmax mixing.
  -> fused PV^T + denominator: lhsT = [v_h | ones] (M=128), PSUM rows 0:64
     accumulate PV^T, rows 64:128 the pre-broadcast denominator
  -> reciprocal (fast approx) + multiply -> outT
  -> row-split out_proj per query block, interleaved into the next block's
     attention so the PE never idles; partial y DMA'd out as it's produced.
"""

import numpy as np
from contextlib import ExitStack

import concourse.tile as tile
from concourse import bacc, mybir
from concourse.bass_utils import run_bass_kernel_spmd

# --------------------------------------------------------------------------
# Custom DVE op: quadratic sawtooth correction for the bit-trick exp2.
# y0 = bitcast(int32(u*2^23 + 127*2^23)) = 2^n * (1+g); this op computes
# y0 * q(r), r = 1+g in [1,2), q(r) ~= 2^(r-1)/r (rel minimax quadratic).
# --------------------------------------------------------------------------
import concourse.dve_ops as _dops
from concourse.dve_spec import (Spec as _Spec, Src0 as _Src0, C0 as _C0,
                                C1 as _C1, C2 as _C2, C3 as _C3, One as _One,
                                AluOp as _AluOp, Bin as _Bin,
                                lower as _lower, _spill_c3_to_src1,
                                _has_src1)
from concourse.dve_uop import DveOpSpec as _DveOpSpec

QC0, QC1, QC2 = 1.4517454385304476, -0.6921680514362218, 0.23303615941790978
MASK_F32 = float(np.int32(0x007FFFFF).view(np.float32))
LN2 = float(np.log(2.0))
LOG2E_8 = float(np.log2(np.e) / 8.0)   # folds the 1/sqrt(D) logit scale


def _make_exp2_op():
    name = "EXP2_SAW_FIX_ANT"
    for o in _dops.OPS:
        if o.name == name:
            return o
    a_n = _Bin(_AluOp.BITWISE_AND, _Src0, _C3)     # C3 (in1): mantissa mask
    r_n = _Bin(_AluOp.BITWISE_OR, a_n, _One)       # r = 1+g in [1,2)
    body = _spill_c3_to_src1(_Src0 * (_C0 + (_C1 + _C2 * r_n) * r_n))

    def _ref(in0, in1, s0, s1, imm2):
        y0 = np.asarray(in0, np.float32)
        aa = y0.view(np.int32) & np.int32(0x007FFFFF)
        rr = (aa | np.int32(0x3F800000)).view(np.float32)
        qq = (np.float32(s0) + (np.float32(s1) + np.float32(imm2) * rr)
              * rr).astype(np.float32)
        return (y0 * qq).astype(np.float32)

    spec = _Spec(body=body, reference=_ref)
    row = _dops._CUSTOM_DVE_ROW_BASE + len(_dops.OPS)
    shas = {}
    for ver in ("v3", "v4"):
        shas[ver] = _DveOpSpec(name=name, opcode=row,
                               uops=_lower(spec, ver=ver),
                               rd1_en=_has_src1(spec)).sha(ver)
    op = _dops.DveOp(name, spec, subdim=False, uops_sha=shas)
    _dops.OPS.append(op)
    _dops.CUSTOM_DVE_SPECS[name] = spec
    _dops._SUB_OPCODE_FOR_NAME[name] = row
    return op


EXP2_OP = _make_exp2_op()

B, S, E, H, D = 2, 2048, 1024, 16, 64
N_CORES = 8
HPC = 4            # heads per core
HD = HPC * D       # 256

F32 = mybir.dt.float32
F32R = mybir.dt.float32r
I32 = mybir.dt.int32

_cached = None


_DVE_KC = ({2, 6, 11}, {4, 8, 13})   # per-hp key chunks routed to VectorE


def _dve_exp(hp, kc, qc):
    """Which exp units run on the Vector engine (2-inst bit trick)."""
    return kc in _DVE_KC[hp]


def build(reps=1, profile=False):
    nc = bacc.Bacc("TRN2", target_bir_lowering=False, debug=False,
                   num_devices=N_CORES)

    xT_d = nc.dram_tensor("xT", [E, S], F32R, kind="ExternalInput").ap()
    wq_d = nc.dram_tensor("wq", [E, HD], F32R, kind="ExternalInput").ap()
    wk_d = nc.dram_tensor("wk", [E, HD], F32R, kind="ExternalInput").ap()
    wv_d = nc.dram_tensor("wv", [E, HD], F32R, kind="ExternalInput").ap()
    wo_d = nc.dram_tensor("wo", [HD, E], F32R, kind="ExternalInput").ap()
    y_d = nc.dram_tensor("y", [S, E], F32, kind="ExternalOutput").ap()

    with tile.TileContext(nc) as tc, ExitStack() as ctx:
        glob = ctx.enter_context(tc.tile_pool(name="glob", bufs=1))
        qT_t = glob.tile([128, 2, S], F32R, tag="qT")    # head-pair dims x S
        kT_t = glob.tile([128, 2, S], F32R, tag="kT")
        v_t = glob.tile([128, 16, HPC, 128], F32R, tag="v")   # [v_h | ones]
        outT_t = glob.tile([128, 2, S], F32R, tag="outT")
        wo_t = glob.tile([128, 2, E], F32R, tag="wo")
        mask_t = glob.tile([128, 1], F32, tag="mask")
        ones_f = glob.tile([128, D], F32, tag="ones_f")

        nc.vector.memset(mask_t[:], MASK_F32)
        nc.gpsimd.memset(ones_f[:], 1.0)
        for sc in range(16):
            for h in range(HPC):
                nc.vector.tensor_copy(v_t[:, sc, h, 64:128], ones_f[:])

        tail = None
        for _rep in range(reps):
            tail = _emit_body(nc, tc, xT_d, wq_d, wk_d, wv_d, wo_d, y_d,
                              qT_t, kT_t, v_t, outT_t, wo_t, mask_t,
                              prelude=tail)
        if tail is not None:
            with tc.tile_pool(name="fpp", bufs=2, space="PSUM") as fpp, \
                 tc.tile_pool(name="fys", bufs=2) as fys:
                tail(fpp, fys)

    nc.compile()
    return nc


def _out_proj_chunk(nc, psum_pool, ysb_pool, outT_t, wo_t, y_d, qc, mc,
                    psum_tag="st"):
    yt = ysb_pool.tile([128, E], F32, tag="y")
    for half in range(2):
        yp = psum_pool.tile([128, 512], F32, tag=psum_tag)
        for hp in range(2):
            nc.tensor.matmul(
                yp[:],
                outT_t[:, hp, qc * 512 + mc * 128:qc * 512 + (mc + 1) * 128],
                wo_t[:, hp, half * 512:(half + 1) * 512],
                start=(hp == 0), stop=(hp == 1))
        dst = yt[:, half * 512:(half + 1) * 512]
        if half == 0:
            nc.vector.tensor_copy(dst, yp[:])
        else:
            nc.scalar.copy(dst, yp[:])
    row = qc * 512 + mc * 128
    nc.sync.dma_start(y_d[row:row + 128, :], yt[:])


def _emit_body(nc, tc, xT_d, wq_d, wk_d, wv_d, wo_d, y_d,
               qT_t, kT_t, v_t, outT_t, wo_t, mask_t, dbg=None,
               prelude=None):
    xT_r = xT_d.rearrange("(c p) s -> p c s", p=128)

    # ---- Phase P: projections (q^T, k^T, v), streamed by 512-wide s-chunk
    with ExitStack() as pb:
        wpool = pb.enter_context(tc.tile_pool(name="wpool", bufs=1))
        xload = pb.enter_context(tc.tile_pool(name="xload", bufs=3))
        ppsum = pb.enter_context(tc.tile_pool(name="ppsum", bufs=2,
                                              space="PSUM"))
        if prelude is not None:
            ypp = pb.enter_context(tc.tile_pool(name="ypp", bufs=2,
                                                space="PSUM"))
            ytl = pb.enter_context(tc.tile_pool(name="ytl", bufs=2))
        wq_t = wpool.tile([128, 8, HD], F32R, tag="wq")
        wk_t = wpool.tile([128, 8, HD], F32R, tag="wk")
        wv_t = wpool.tile([128, 8, HD], F32R, tag="wv")
        # DMA order: get the first q-projection started as early as possible;
        # wo is not needed until the first out_proj (~80us in)
        nc.sync.dma_start(wq_t[:], wq_d.rearrange("(c p) n -> p c n", p=128))
        xcs = []
        xc = xload.tile([128, 8, 512], F32R, tag="xc")
        nc.sync.dma_start(xc[:], xT_r[:, :, 0:512])
        xcs.append(xc)
        nc.sync.dma_start(wk_t[:], wk_d.rearrange("(c p) n -> p c n", p=128))
        nc.sync.dma_start(wv_t[:], wv_d.rearrange("(c p) n -> p c n", p=128))
        for sq in range(1, 4):
            xc = xload.tile([128, 8, 512], F32R, tag="xc")
            nc.sync.dma_start(xc[:], xT_r[:, :, sq * 512:(sq + 1) * 512])
            xcs.append(xc)
        nc.sync.dma_start(wo_t[:], wo_d.rearrange("(c p) n -> p c n", p=128))

        for sq in range(4):
            xc = xcs[sq]
            for w_t, dst in ((wq_t, qT_t), (wk_t, kT_t)):
                for mc in range(2):
                    pp = ppsum.tile([128, 512], F32, tag="pp")
                    for ec in range(8):
                        nc.tensor.matmul(
                            pp[:],
                            w_t[:, ec, mc * 128:(mc + 1) * 128],
                            xc[:, ec, :],
                            start=(ec == 0), stop=(ec == 7))
                    nc.vector.tensor_copy(
                        dst[:, mc, sq * 512:(sq + 1) * 512], pp[:])
            for scl in range(4):
                sc = sq * 4 + scl
                vp = ppsum.tile([128, HD], F32, tag="vp")
                for ec in range(8):
                    nc.tensor.matmul(
                        vp[:],
                        xc[:, ec, scl * 128:(scl + 1) * 128],
                        wv_t[:, ec, :],
                        start=(ec == 0), stop=(ec == 7))
                nc.vector.tensor_copy(
                    v_t[:, sc, :, 0:64],
                    vp[:].rearrange("p (h d) -> p h d", h=HPC))
            if sq == 0 and prelude is not None:
                prelude(ypp, ytl)

    # ---- Phase A: attention + interleaved row-split out_proj ----
    with ExitStack() as ab:
        stp = ab.enter_context(tc.tile_pool(name="stp", bufs=2, space="PSUM"))
        pvp = ab.enter_context(tc.tile_pool(name="pvp", bufs=2, space="PSUM"))
        expp = ab.enter_context(tc.tile_pool(name="expp", bufs=6))
        seedp = ab.enter_context(tc.tile_pool(name="seedp", bufs=3))
        rcp = ab.enter_context(tc.tile_pool(name="rcp", bufs=4))
        ysb = ab.enter_context(tc.tile_pool(name="ysb", bufs=3))

        def out_proj_chunk(qc, mc):
            _out_proj_chunk(nc, stp, ysb, outT_t, wo_t, y_d, qc, mc)

        for qc in range(4):
            fz0 = pvp.tile([128, 2, 512], F32, tag="pv")
            fz1 = pvp.tile([128, 2, 512], F32, tag="pv")
            fzs = [fz0, fz1]

            def emit_pv(hp, kc, ex):
                for par in range(2):
                    h = 2 * hp + par
                    nc.tensor.matmul(
                        fzs[hp][:, par, :],
                        v_t[:, kc, h, :],
                        ex[:, par * 512:(par + 1) * 512],
                        start=(kc == 0), stop=(kc == 15))

            def finish_hp(hp):
                # normalization for this head pair, as soon as its PV is done
                for par in range(2):
                    rc = rcp.tile([64, 512], F32, tag="rc")
                    nc.vector.reciprocal(rc[:], fzs[hp][64:128, par, :])
                    nc.vector.tensor_mul(
                        outT_t[par * 64:(par + 1) * 64, hp,
                               qc * 512:(qc + 1) * 512],
                        fzs[hp][0:64, par, :], rc[:])

            pend = []   # PV lagged 2 units behind S^T/exp to hide exp latency
            for kc in range(16):
                for hp in range(2):
                    st = stp.tile([128, 1024], F32, tag="st")
                    for par in range(2):     # row-packed K=64 head pair
                        lo, hi = par * 64, (par + 1) * 64
                        nc.tensor.matmul(
                            st[:, par * 512:(par + 1) * 512],
                            kT_t[lo:hi, hp, kc * 128:(kc + 1) * 128],
                            qT_t[lo:hi, hp, qc * 512:(qc + 1) * 512],
                            start=True, stop=True)
                    ex = expp.tile([128, 1024], F32R, tag="ex")
                    if _dve_exp(hp, kc, qc):
                        seed = seedp.tile([128, 1024], I32, tag="seed")
                        nc.vector.tensor_scalar(
                            seed[:], st[:],
                            8388608.0, 1065353216.0,
                            mybir.AluOpType.mult, mybir.AluOpType.add)
                        nc.vector._custom_dve(
                            EXP2_OP, out=ex[:], in0=seed[:].bitcast(F32),
                            in1=mask_t[:], s0=QC0, s1=QC1, imm2=QC2)
                    else:
                        nc.scalar.activation(
                            ex[:], st[:],
                            mybir.ActivationFunctionType.Exp, scale=LN2)
                    if dbg is not None and qc == 0 and kc == 0:
                        nc.sync.dma_start(dbg["ex"][hp], ex[:])
                    pend.append((hp, kc, ex))
                    if len(pend) > 2:
                        emit_pv(*pend.pop(0))
                if qc >= 1 and kc % 4 == 3:
                    out_proj_chunk(qc - 1, kc // 4)
            for item in pend:
                emit_pv(*item)
                if item[1] == 15:
                    finish_hp(item[0])
        if dbg is not None:
            nc.sync.dma_start(dbg["qT"], qT_t[:])
            nc.sync.dma_start(dbg["kT"], kT_t[:])
            nc.sync.dma_start(dbg["v"], v_t[:])
            nc.sync.dma_start(dbg["outT"], outT_t[:])

    def tail(psum_pool, ysb_pool):
        # out_proj for the last query block, folded into the next rep's
        # projection phase so its PSUM frees before the next rep needs it
        for mc in range(4):
            _out_proj_chunk(nc, psum_pool, ysb_pool, outT_t, wo_t, y_d,
                            3, mc, psum_tag="yp")
    return tail


def _get_nc():
    global _cached
    if _cached is None:
        _cached = build()
    return _cached


def make_in_maps(x, w_qkv, w_out):
    x = np.asarray(x, dtype=np.float32)
    w_qkv = np.asarray(w_qkv, dtype=np.float32)
    w_out = np.asarray(w_out, dtype=np.float32)
    scale = np.float32(LOG2E_8)
    in_maps = []
    for c in range(N_CORES):
        b, r = c // 4, c % 4
        hs = r * HD
        in_maps.append({
            "xT": np.ascontiguousarray(x[b].T),
            "wq": np.ascontiguousarray(w_qkv[:, hs:hs + HD] * scale),
            "wk": np.ascontiguousarray(w_qkv[:, E + hs:E + hs + HD]),
            "wv": np.ascontiguousarray(w_qkv[:, 2 * E + hs:2 * E + hs + HD]),
            "wo": np.ascontiguousarray(w_out[hs:hs + HD, :]),
        })
    return in_maps


def assemble(results):
    y = np.zeros((B, S, E), dtype=np.float32)
    for c in range(N_CORES):
        y[c // 4] += results[c]["y"]
    return y


def kernel(x, w_qkv, w_out):
    nc = _get_nc()
    res = run_bass_kernel_spmd(nc, make_in_maps(x, w_qkv, w_out),
                               list(range(N_CORES)))
    return assemble(res.results)
